# revision 27
# baseline (speedup 1.0000x reference)
"""BalancedMoE (B=8192, D=2048, E=8, top-2) on 8 Trainium2 NeuronCores.

Strategy: expert-parallel with host-side sparse dispatch.
  - Host computes gate logits / top-2 routing / softmax gates (tiny:
    8192x2048 @ 2048x8), gathers each expert's tokens, and transposes
    them into [D, C] so the device needs no on-chip transposes.
  - Core e runs a dense [C, D] x [D, D] matmul for expert e only
    (top-2 of 8 experts => 4x less FLOPs than the dense reference),
    with the expert weight matrix stationary in SBUF.
  - Host scatters the per-expert outputs back and combines with the
    gate weights.

Per-core Bass kernel: outT[o, t] = sum_d W_e[o, d] * toks[t, d] + b_e[o]
  lhsT = W_e^T tiles (stationary), rhs = toksT tiles (moving).
"""

import os

import numpy as np

P = 128
B = 8192
D_LAT = 1024
D_EMB = 1024
D = D_LAT + D_EMB  # 2048
E = 8
TOPK = 2
N_CORES = 8


# ----------------------------------------------------------------- device ---

_cache = {}


def _ntff_shim():
    """Register the axon NTFF profile hook that the boot skips when
    antenv.axon_hooks is missing (so BASS_TRACE=1 yields exec_time_ns)."""
    import sys
    import types

    if "antenv.axon_hooks" in sys.modules:
        return
    holder = [None]
    mod = types.ModuleType("antenv.axon_hooks")
    mod.set_axon_ntff_profile_hook = lambda h: holder.__setitem__(0, h)
    mod.get_axon_ntff_profile_hook = lambda: holder[0]
    sys.modules["antenv.axon_hooks"] = mod
    try:
        import antenv

        antenv.axon_hooks = mod
        from trn_agent_boot.trn_boot import _ntff_profile_via_ctypes

        mod.set_axon_ntff_profile_hook(
            _ntff_profile_via_ctypes("/opt/axon/libaxon_pjrt.so")
        )
    except Exception:
        pass


def _n_tiles(C):
    """Split C into moving-operand tiles of width 256..512 (float32r needs
    >=256 columns per matmul for full PE rate; PSUM caps a tile at 512).
    Large tiles first so the fetch-bound start phase has dense PE work."""
    assert C >= 512
    k = (C - 256) // 512 if C % 512 else C // 512
    rem = C - 512 * k
    sizes = [512] * k
    if rem == 0:
        pass
    elif rem <= 512:
        sizes.append(rem)
    else:  # 513..767: two tiles, both >= 256
        sizes.extend([rem - 256, 256])
    return sizes


def _build(C, dt_name):
    import concourse.mybir as mybir
    from concourse import bacc
    from concourse.bass import ds
    from concourse.tile import TileContext

    dt_in = getattr(mybir.dt, dt_name)
    KT = D // P
    MT = D // P
    n_sizes = _n_tiles(C)
    nc = bacc.Bacc(
        "TRN2", target_bir_lowering=False, debug=False, num_devices=N_CORES
    )
    # wp[m, ki, ko, o] = W_e[m*128 + o, ko*128 + ki] — per-m chunks are
    # contiguous so each weight-chunk DMA moves 8KB/partition runs.
    wp = nc.dram_tensor("wp", [MT, P, KT, P], dt_in, kind="ExternalInput")
    toksT = nc.dram_tensor("toksT", [D, C], dt_in, kind="ExternalInput")
    bias = nc.dram_tensor("bias", [D], mybir.dt.float32, kind="ExternalInput")
    outT = nc.dram_tensor("outT", [D, C], mybir.dt.float32, kind="ExternalOutput")

    t_r = toksT.ap().rearrange("(ko ki) n -> ki ko n", ki=P)
    o_r = outT.ap().rearrange("(mo mi) n -> mi mo n", mi=P)
    b_r = bias.ap().rearrange("(mo mi) -> mi mo", mi=P)

    with TileContext(nc) as tc:
        with (
            tc.tile_pool(name="w", bufs=1) as w_pool,
            tc.tile_pool(name="tok", bufs=2) as tok_pool,
            tc.tile_pool(name="out", bufs=3) as out_pool,
            tc.tile_pool(name="bias", bufs=1) as b_pool,
            tc.tile_pool(name="ps", bufs=8, space="PSUM") as ps_pool,
        ):
            bias_tile = b_pool.tile([P, MT], mybir.dt.float32)
            nc.sync.dma_start(bias_tile[:], b_r)
            # DMA issue order = consumption order: first token tile, then
            # per-m weight chunks (compute starts as soon as chunk 0 lands
            # instead of waiting for the whole 16.8MB weight load).
            tok_tiles = {}

            def load_toks(n, n_off, n_sz, chunked=False):
                t_full = tok_pool.tile([P, KT, 512], dt_in, tag="tok")
                t_tile = t_full[:, :, :n_sz]
                if chunked:
                    # per-k chunks so the first matmuls only wait for the
                    # k-slices they read, not the whole 4.2MB tile
                    for k in range(KT):
                        nc.sync.dma_start(
                            t_tile[:, k : k + 1, :],
                            t_r[:, k : k + 1, ds(n_off, n_sz)],
                        )
                else:
                    nc.sync.dma_start(t_tile, t_r[:, :, ds(n_off, n_sz)])
                tok_tiles[n] = t_tile

            w_tiles = [None] * MT

            def load_w(m):
                w_t = w_pool.tile([P, KT, P], dt_in, tag=f"w{m}")
                # weights ride the Activation-HWDGE queues; tokens/outputs
                # ride SP-HWDGE, so the streams don't interleave in one queue
                nc.scalar.dma_start(w_t[:], wp.ap()[m])
                w_tiles[m] = w_t

            # issue order ~= consumption order: w0, first token tile
            # (k-chunked), remaining weight chunks, then the n=1 token
            # prefetch behind the weight stream.
            load_w(0)
            load_toks(0, 0, n_sizes[0], chunked=True)
            for m in range(1, MT):
                load_w(m)
            if len(n_sizes) > 1:
                load_toks(1, n_sizes[0], n_sizes[1])

            n_off = 0
            for n, n_sz in enumerate(n_sizes):
                if n > 1:
                    load_toks(n, n_off, n_sz)
                t_tile = tok_tiles.pop(n)
                for m in range(MT):
                    ps_full = ps_pool.tile([P, 512], mybir.dt.float32, tag="ps")
                    ps = ps_full[:, :n_sz]
                    for k in range(KT):
                        nc.tensor.matmul(
                            ps,
                            w_tiles[m][:, k, :],
                            t_tile[:, k, :],
                            start=(k == 0),
                            stop=(k == KT - 1),
                        )
                    o_full = out_pool.tile([P, 512], mybir.dt.float32, tag="out")
                    o_tile = o_full[:, :n_sz]
                    nc.vector.tensor_scalar_add(
                        o_tile, ps, bias_tile[:, m : m + 1]
                    )
                    nc.sync.dma_start(o_r[:, m, ds(n_off, n_sz)], o_tile)
                n_off += n_sz
    nc.compile()
    return nc


def _get_program(C, dt_name):
    key = (C, dt_name)
    if key not in _cache:
        _cache[key] = _build(C, dt_name)
    return _cache[key]


# ------------------------------------------------------------------- host ---


def kernel(x, y, W_experts, b_experts, W_gate, b_gate):
    x = np.asarray(x, dtype=np.float32)
    y = np.asarray(y, dtype=np.float32)
    W_experts = np.asarray(W_experts, dtype=np.float32)
    b_experts = np.asarray(b_experts, dtype=np.float32)
    W_gate = np.asarray(W_gate, dtype=np.float32)
    b_gate = np.asarray(b_gate, dtype=np.float32)

    inp = np.concatenate([x, y], axis=1)  # [B, D]

    # ---- routing (host) ----
    logits = inp.astype(np.float64) @ W_gate.T.astype(np.float64) + b_gate
    order = np.argsort(-logits, axis=1, kind="stable")
    top2 = order[:, :TOPK]  # [B, 2]
    v = np.take_along_axis(logits, top2, axis=1)
    v = v - v.max(axis=1, keepdims=True)
    ev = np.exp(v)
    g = (ev / ev.sum(axis=1, keepdims=True)).astype(np.float32)  # [B, 2]

    counts = np.bincount(top2.ravel(), minlength=E)
    # exact capacity (any tile width >= 256 runs at full f32r rate, so no
    # rounding needed beyond the 512 floor)
    C = max(512, int(counts.max()))

    idx_list = []
    wgt_list = []
    for e in range(E):
        m0 = top2[:, 0] == e
        m1 = top2[:, 1] == e
        idx_e = np.concatenate([np.nonzero(m0)[0], np.nonzero(m1)[0]])
        w_e = np.concatenate([g[m0, 0], g[m1, 1]])
        idx_list.append(idx_e)
        wgt_list.append(w_e)

    dt_name = os.environ.get("MOE_DT", "float32r")
    if dt_name == "bfloat16":
        import ml_dtypes

        np_in_dt = np.dtype(ml_dtypes.bfloat16)
    else:
        np_in_dt = np.dtype(np.float32)

    inpT = np.ascontiguousarray(inp.T)  # [D, B]
    MT = KT = D // P
    in_maps = []
    for e in range(E):
        toksT = np.zeros((D, C), dtype=np_in_dt)
        toksT[:, : len(idx_list[e])] = inpT[:, idx_list[e]].astype(np_in_dt)
        # wp[m, ki, ko, o] = W_e[m*128 + o, ko*128 + ki]
        wp = np.ascontiguousarray(
            W_experts[e].reshape(MT, P, KT, P).transpose(0, 3, 2, 1).astype(np_in_dt)
        )
        in_maps.append({"wp": wp, "toksT": toksT, "bias": b_experts[e]})

    # ---- device ----
    if os.environ.get("BASS_TRACE"):
        _ntff_shim()
    from concourse.bass_utils import run_bass_kernel_spmd

    nc = _get_program(C, dt_name)
    res = None
    for attempt in range(3):
        try:
            res = run_bass_kernel_spmd(nc, in_maps, core_ids=list(range(N_CORES)))
            break
        except Exception:
            # the axon-tunneled device occasionally reports a transient
            # NRT_EXEC_UNIT_UNRECOVERABLE; it recovers after a short wait
            if attempt == 2:
                raise
            import time

            time.sleep(20 * (attempt + 1))
            try:
                import jax

                jax.clear_caches()
            except Exception:
                pass
    globals()["_last_res"] = res
    if res.exec_time_ns is not None:
        print(f"HW exec time: {res.exec_time_ns} ns")

    # ---- combine (host) ----
    fused = np.zeros((B, D), dtype=np.float32)
    for e in range(E):
        n_e = len(idx_list[e])
        if n_e == 0:
            continue
        out_rows = res.results[e]["outT"][:, :n_e].T  # [n_e, D]
        fused[idx_list[e]] += out_rows * wgt_list[e][:, None]
    return fused



# revision 28
# speedup vs baseline: 1.0401x; 1.0401x over previous
"""BalancedMoE (B=8192, D=2048, E=8, top-2) on 8 Trainium2 NeuronCores.

Strategy: expert-parallel with host-side sparse dispatch.
  - Host computes gate logits / top-2 routing / softmax gates (tiny:
    8192x2048 @ 2048x8), gathers each expert's tokens, and transposes
    them into [D, C] so the device needs no on-chip transposes.
  - Core e runs a dense [C, D] x [D, D] matmul for expert e only
    (top-2 of 8 experts => 4x less FLOPs than the dense reference),
    with the expert weight matrix stationary in SBUF.
  - Host scatters the per-expert outputs back and combines with the
    gate weights.

Per-core Bass kernel: outT[o, t] = sum_d W_e[o, d] * toks[t, d] + b_e[o]
  lhsT = W_e^T tiles (stationary), rhs = toksT tiles (moving).
"""

import os

import numpy as np

P = 128
B = 8192
D_LAT = 1024
D_EMB = 1024
D = D_LAT + D_EMB  # 2048
E = 8
TOPK = 2
N_CORES = 8


# ----------------------------------------------------------------- device ---

_cache = {}


def _ntff_shim():
    """Register the axon NTFF profile hook that the boot skips when
    antenv.axon_hooks is missing (so BASS_TRACE=1 yields exec_time_ns)."""
    import sys
    import types

    if "antenv.axon_hooks" in sys.modules:
        return
    holder = [None]
    mod = types.ModuleType("antenv.axon_hooks")
    mod.set_axon_ntff_profile_hook = lambda h: holder.__setitem__(0, h)
    mod.get_axon_ntff_profile_hook = lambda: holder[0]
    sys.modules["antenv.axon_hooks"] = mod
    try:
        import antenv

        antenv.axon_hooks = mod
        from trn_agent_boot.trn_boot import _ntff_profile_via_ctypes

        mod.set_axon_ntff_profile_hook(
            _ntff_profile_via_ctypes("/opt/axon/libaxon_pjrt.so")
        )
    except Exception:
        pass


def _n_tiles(C):
    """Split C into moving-operand tiles of width 256..512 (float32r needs
    >=256 columns per matmul for full PE rate; PSUM caps a tile at 512).
    Large tiles first so the fetch-bound start phase has dense PE work."""
    assert C >= 512
    k = (C - 256) // 512 if C % 512 else C // 512
    rem = C - 512 * k
    sizes = [512] * k
    if rem == 0:
        pass
    elif rem <= 512:
        sizes.append(rem)
    else:  # 513..767: two tiles, both >= 256
        sizes.extend([rem - 256, 256])
    return sizes


def _build(C, dt_name):
    import concourse.mybir as mybir
    from concourse import bacc
    from concourse.bass import ds
    from concourse.tile import TileContext

    dt_in = getattr(mybir.dt, dt_name)
    KT = D // P
    MT = D // P
    n_sizes = _n_tiles(C)
    nc = bacc.Bacc(
        "TRN2", target_bir_lowering=False, debug=False, num_devices=N_CORES
    )
    # wp[m, ki, ko, o] = W_e[m*128 + o, ko*128 + ki] — per-m chunks are
    # contiguous so each weight-chunk DMA moves 8KB/partition runs.
    wp = nc.dram_tensor("wp", [MT, P, KT, P], dt_in, kind="ExternalInput")
    toksT = nc.dram_tensor("toksT", [D, C], dt_in, kind="ExternalInput")
    bias = nc.dram_tensor("bias", [D], mybir.dt.float32, kind="ExternalInput")
    outT = nc.dram_tensor("outT", [D, C], mybir.dt.float32, kind="ExternalOutput")

    t_r = toksT.ap().rearrange("(ko ki) n -> ki ko n", ki=P)
    o_r = outT.ap().rearrange("(mo mi) n -> mi mo n", mi=P)
    b_r = bias.ap().rearrange("(mo mi) -> mi mo", mi=P)

    with TileContext(nc) as tc:
        with (
            tc.tile_pool(name="w", bufs=1) as w_pool,
            tc.tile_pool(name="tok", bufs=2) as tok_pool,
            tc.tile_pool(name="out", bufs=3) as out_pool,
            tc.tile_pool(name="bias", bufs=1) as b_pool,
            tc.tile_pool(name="ps", bufs=8, space="PSUM") as ps_pool,
        ):
            bias_tile = b_pool.tile([P, MT], mybir.dt.float32)
            nc.sync.dma_start(bias_tile[:], b_r)
            # DMA issue order = consumption order: first token tile, then
            # per-m weight chunks (compute starts as soon as chunk 0 lands
            # instead of waiting for the whole 16.8MB weight load).
            tok_tiles = {}

            def load_toks(n, n_off, n_sz, chunked=False):
                t_full = tok_pool.tile([P, KT, 512], dt_in, tag="tok")
                t_tile = t_full[:, :, :n_sz]
                if chunked:
                    # per-k chunks so the first matmuls only wait for the
                    # k-slices they read, not the whole 4.2MB tile
                    for k in range(KT):
                        nc.sync.dma_start(
                            t_tile[:, k : k + 1, :],
                            t_r[:, k : k + 1, ds(n_off, n_sz)],
                        )
                else:
                    nc.sync.dma_start(t_tile, t_r[:, :, ds(n_off, n_sz)])
                tok_tiles[n] = t_tile

            w_tiles = [None] * MT

            def load_w(m):
                w_t = w_pool.tile([P, KT, P], dt_in, tag=f"w{m}")
                # weights ride the Activation-HWDGE queues; tokens/outputs
                # ride SP-HWDGE, so the streams don't interleave in one queue
                nc.scalar.dma_start(w_t[:], wp.ap()[m])
                w_tiles[m] = w_t

            # issue order ~= consumption order: w0, first token tile
            # (k-chunked), remaining weight chunks, then the n=1 token
            # prefetch behind the weight stream.
            load_w(0)
            load_toks(0, 0, n_sizes[0], chunked=True)
            for m in range(1, MT):
                load_w(m)

            n_off = 0
            for n, n_sz in enumerate(n_sizes):
                if n > 1:
                    load_toks(n, n_off, n_sz)
                t_tile = tok_tiles.pop(n)
                for m in range(MT):
                    # issue the n=1 token prefetch from the middle of n=0's
                    # SP stream: early enough to land before n=1 starts,
                    # late enough not to steal HBM from the weight stream
                    if n == 0 and m == 8 and len(n_sizes) > 1:
                        load_toks(1, n_sizes[0], n_sizes[1])
                    ps_full = ps_pool.tile([P, 512], mybir.dt.float32, tag="ps")
                    ps = ps_full[:, :n_sz]
                    for k in range(KT):
                        nc.tensor.matmul(
                            ps,
                            w_tiles[m][:, k, :],
                            t_tile[:, k, :],
                            start=(k == 0),
                            stop=(k == KT - 1),
                        )
                    o_full = out_pool.tile([P, 512], mybir.dt.float32, tag="out")
                    o_tile = o_full[:, :n_sz]
                    nc.vector.tensor_scalar_add(
                        o_tile, ps, bias_tile[:, m : m + 1]
                    )
                    nc.sync.dma_start(o_r[:, m, ds(n_off, n_sz)], o_tile)
                n_off += n_sz
    nc.compile()
    return nc


def _get_program(C, dt_name):
    key = (C, dt_name)
    if key not in _cache:
        _cache[key] = _build(C, dt_name)
    return _cache[key]


# ------------------------------------------------------------------- host ---


def kernel(x, y, W_experts, b_experts, W_gate, b_gate):
    x = np.asarray(x, dtype=np.float32)
    y = np.asarray(y, dtype=np.float32)
    W_experts = np.asarray(W_experts, dtype=np.float32)
    b_experts = np.asarray(b_experts, dtype=np.float32)
    W_gate = np.asarray(W_gate, dtype=np.float32)
    b_gate = np.asarray(b_gate, dtype=np.float32)

    inp = np.concatenate([x, y], axis=1)  # [B, D]

    # ---- routing (host) ----
    logits = inp.astype(np.float64) @ W_gate.T.astype(np.float64) + b_gate
    order = np.argsort(-logits, axis=1, kind="stable")
    top2 = order[:, :TOPK]  # [B, 2]
    v = np.take_along_axis(logits, top2, axis=1)
    v = v - v.max(axis=1, keepdims=True)
    ev = np.exp(v)
    g = (ev / ev.sum(axis=1, keepdims=True)).astype(np.float32)  # [B, 2]

    counts = np.bincount(top2.ravel(), minlength=E)
    # exact capacity (any tile width >= 256 runs at full f32r rate, so no
    # rounding needed beyond the 512 floor)
    C = max(512, int(counts.max()))

    idx_list = []
    wgt_list = []
    for e in range(E):
        m0 = top2[:, 0] == e
        m1 = top2[:, 1] == e
        idx_e = np.concatenate([np.nonzero(m0)[0], np.nonzero(m1)[0]])
        w_e = np.concatenate([g[m0, 0], g[m1, 1]])
        idx_list.append(idx_e)
        wgt_list.append(w_e)

    dt_name = os.environ.get("MOE_DT", "float32r")
    if dt_name == "bfloat16":
        import ml_dtypes

        np_in_dt = np.dtype(ml_dtypes.bfloat16)
    else:
        np_in_dt = np.dtype(np.float32)

    inpT = np.ascontiguousarray(inp.T)  # [D, B]
    MT = KT = D // P
    in_maps = []
    for e in range(E):
        toksT = np.zeros((D, C), dtype=np_in_dt)
        toksT[:, : len(idx_list[e])] = inpT[:, idx_list[e]].astype(np_in_dt)
        # wp[m, ki, ko, o] = W_e[m*128 + o, ko*128 + ki]
        wp = np.ascontiguousarray(
            W_experts[e].reshape(MT, P, KT, P).transpose(0, 3, 2, 1).astype(np_in_dt)
        )
        in_maps.append({"wp": wp, "toksT": toksT, "bias": b_experts[e]})

    # ---- device ----
    if os.environ.get("BASS_TRACE"):
        _ntff_shim()
    from concourse.bass_utils import run_bass_kernel_spmd

    nc = _get_program(C, dt_name)
    res = None
    for attempt in range(3):
        try:
            res = run_bass_kernel_spmd(nc, in_maps, core_ids=list(range(N_CORES)))
            break
        except Exception:
            # the axon-tunneled device occasionally reports a transient
            # NRT_EXEC_UNIT_UNRECOVERABLE; it recovers after a short wait
            if attempt == 2:
                raise
            import time

            time.sleep(20 * (attempt + 1))
            try:
                import jax

                jax.clear_caches()
            except Exception:
                pass
    globals()["_last_res"] = res
    if res.exec_time_ns is not None:
        print(f"HW exec time: {res.exec_time_ns} ns")

    # ---- combine (host) ----
    fused = np.zeros((B, D), dtype=np.float32)
    for e in range(E):
        n_e = len(idx_list[e])
        if n_e == 0:
            continue
        out_rows = res.results[e]["outT"][:, :n_e].T  # [n_e, D]
        fused[idx_list[e]] += out_rows * wgt_list[e][:, None]
    return fused

